# revision 3
# baseline (speedup 1.0000x reference)
"""Trainium2 Bass kernel: int8 3x3 VALID conv (1,512,512,32)->(1,510,510,64)
with TFLite fixed-point requantization, SPMD over 8 NeuronCores (output rows).

v3 design:
- x is packed + converted to bf16 ON HOST into per-block SBUF-shaped slabs
  [128 = 4 row-phases x 32 cin, 4096 = {A,B} x 4 groups x 512], so device
  DMAs are big contiguous reads and the DVE does ZERO convert work.
- Per half-group: one psum tile [128,1024] (2 banks); 6 matmuls ordered
  j-outer so consecutive matmuls share the same stationary weights
  (A-tap-j then B-tap-j), enabling weight-load reuse on hardware.
- Requant (per-channel out = sat(round(acc*sc + zb))) split across DVE
  (tensor_scalar) and ACT (activation) -- both round-to-nearest on HW.
- x/w DMAs issue on SP+Pool+DVE queues in parallel for a short head; out
  DMAs issue on the otherwise-idle Pool queue; the final half-group is
  requanted and stored in two halves to shorten the tail.
"""
import numpy as np
import ml_dtypes

import concourse.mybir as mybir
import concourse.tile as tile_mod
import concourse.bacc as bacc
from concourse.bass_utils import run_bass_kernel_spmd
from concourse.tile import TileContext
from concourse.vector_clock import ScopedClock


def _patched_drain_and_barrier(self, tick_clock, wait_clock):
    # workaround: split the Tile kernel-tail drain into single-wait drains
    # (1 sync-wait per CTRL inst), distributed round-robin across engine
    # queues so they wait in parallel instead of serializing on SP.
    drain_inst = self.nc.sync.drain()
    wait_clock.add_sem_waits(
        drain_inst.ins, ScopedClock({None: tick_clock.global_clock})
    )
    si = drain_inst.ins.sync_info
    if si is not None and si.on_wait and len(si.on_wait) > 1:
        waits = list(si.on_wait)
        drain_inst.ins.sync_info = mybir.SyncInfo(
            on_wait=[waits[0]], on_update=si.on_update
        )
        engines = [self.nc.sync, self.nc.gpsimd, self.nc.vector,
                   self.nc.scalar, self.nc.tensor]
        for i, w in enumerate(waits[1:]):
            d2 = engines[i % len(engines)].drain()
            d2.ins.sync_info = mybir.SyncInfo(on_wait=[w], on_update=[])

    self.nc.all_engine_barrier()
    assert self.sems is not None
    popped = self.nc._tile_sem_poison_stack.pop()
    assert popped is self._sem_poison
    self.nc.clear_and_free_semaphores(list(self.sems.allocated().values()))
    self.nc.all_engine_barrier()


tile_mod.TileContext._drain_and_barrier = _patched_drain_and_barrier

dt = mybir.dt
AF = mybir.ActivationFunctionType
OP = mybir.AluOpType

MANT_MAX = 2147418112
H, W, CIN, COUT = 512, 512, 32, 64
WO = 510                     # output width
RC = 64                      # out rows per core
XROWS = 66                   # x rows per core (64 + 2 halo)
NBLK = 4                     # row blocks per core (16 out rows each)
NHG = 16                     # half-groups per core (4 out rows each)

# which engine requants each half-group: 'v' = DVE, 'a' = ACT
REQ_ENG = 'vvavvavvavvavvav'


def build_nc(n_cores: int):
    nc = bacc.Bacc('TRN2', target_bir_lowering=False, debug=False,
                   num_devices=n_cores)
    # xslab[b][32q+c][pk*2048 + g*512 + w] = x[16b + 4g + q + 2*pk][w][c]
    xslab = nc.dram_tensor('xslab', [NBLK, 128, 4096], dt.bfloat16,
                           kind='ExternalInput')
    wgt = nc.dram_tensor('wgt', [128, 3 * 128], dt.bfloat16, kind='ExternalInput')
    qc = nc.dram_tensor('qc', [128, 2], dt.float32, kind='ExternalInput')
    out = nc.dram_tensor('out', [NHG, 128, 2 * WO], dt.int8, kind='ExternalOutput')

    with TileContext(nc) as tc:
        with (
            tc.tile_pool(name='const', bufs=1) as cpool,
            tc.tile_pool(name='xs', bufs=2) as xspool,
            tc.tile_pool(name='ot', bufs=4) as opool,
            tc.tile_pool(name='psum', bufs=4, space='PSUM') as ppool,
        ):
            wsb = cpool.tile([128, 3 * 128], dt.bfloat16)
            qsb = cpool.tile([128, 2], dt.float32)
            q_sc, q_zb = qsb[:, 0:1], qsb[:, 1:2]

            for b in range(NBLK):
                tab = xspool.tile([128, 4096], dt.bfloat16, tag='tab')
                if b == 0:
                    # parallel-issue the critical head DMAs on 4 queues
                    nc.sync.dma_start(wsb[:], wgt[:])
                    nc.gpsimd.dma_start(tab[:, 0:512], xslab[0, :, 0:512])
                    nc.gpsimd.dma_start(tab[:, 2048:2560], xslab[0, :, 2048:2560])
                    nc.scalar.dma_start(qsb[:], qc[:])
                    nc.sync.dma_start(tab[:, 512:2048], xslab[0, :, 512:2048])
                    nc.sync.dma_start(tab[:, 2560:4096], xslab[0, :, 2560:4096])
                else:
                    nc.sync.dma_start(tab[:], xslab[b])

                for hg in range(4):
                    hgi = 4 * b + hg
                    psum = ppool.tile([128, 1024], dt.float32)
                    ot = opool.tile([128, 2 * WO], dt.int8, tag='ot')
                    for j in range(3):
                        for pk in range(2):       # A (h=4hg), B (h=4hg+2)
                            base = pk * 2048 + hg * 512 + j
                            nc.tensor.matmul(
                                psum[:, pk * 512: pk * 512 + WO],
                                wsb[:, j * 128:(j + 1) * 128],
                                tab[:, base: base + WO],
                                start=(j == 0), stop=(j == 2))
                    acc = psum[:].rearrange("p (g w) -> p g w", w=512)[:, :, 0:WO]
                    o3 = ot[:].rearrange("p (g w) -> p g w", w=WO)
                    if hgi == NHG - 1:
                        # split the last half-group so the tail overlaps
                        nc.scalar.activation(o3[:, 0:1], acc[:, 0:1],
                                             AF.Identity, bias=q_zb, scale=q_sc)
                        nc.gpsimd.dma_start(out[hgi, :, 0:WO], ot[:, 0:WO])
                        nc.vector.tensor_scalar(o3[:, 1:2], acc[:, 1:2],
                                                q_sc, q_zb,
                                                op0=OP.mult, op1=OP.add)
                        nc.gpsimd.dma_start(out[hgi, :, WO:2 * WO],
                                            ot[:, WO:2 * WO])
                    else:
                        if REQ_ENG[hgi] == 'v':
                            nc.vector.tensor_scalar(o3, acc, q_sc, q_zb,
                                                    op0=OP.mult, op1=OP.add)
                        else:
                            nc.scalar.activation(o3, acc, AF.Identity,
                                                 bias=q_zb, scale=q_sc)
                        nc.gpsimd.dma_start(out[hgi], ot[:])
    nc.finalize()
    return nc


def host_prepare(x, filt, bias, q_mantissa, exponent, output_zero_point):
    """Full inputs -> list of per-core in_maps."""
    bf16 = ml_dtypes.bfloat16
    x = np.asarray(x)
    filt = np.asarray(filt)
    bias64 = np.asarray(bias).astype(np.int64)
    qm64 = np.asarray(q_mantissa).astype(np.int64)
    ex64 = np.asarray(exponent).astype(np.int64)
    zp = int(np.asarray(output_zero_point))

    # xT: [rows, C, W] bf16, padded to 8*64+2 rows
    xpad = np.zeros((8 * RC + 2, CIN, W), dtype=bf16)
    xpad[:H] = np.ascontiguousarray(x[0].transpose(0, 2, 1)).astype(bf16)

    # weights: wgt[32q+ci, j, 64a+co] = filt[co, q-a, j, ci] (0 <= q-a <= 2)
    wgtf = np.zeros((128, 3, 128), dtype=np.float32)
    for q in range(4):
        for a in range(2):
            fh = q - a
            if 0 <= fh <= 2:
                wgtf[32 * q:32 * q + 32, :, 64 * a:64 * a + 64] = \
                    filt[:, fh, :, :].transpose(2, 1, 0).astype(np.float32)
    wgt_b = np.ascontiguousarray(wgtf.reshape(128, 384)).astype(bf16)

    # per-channel requant constants (f64 -> f32)
    m = np.where(qm64 < MANT_MAX, (qm64 + (1 << 15)) >> 16, 32767).astype(np.float64)
    s = (15 - ex64).astype(np.float64)
    sc = m * (2.0 ** -s)
    zb = zp + bias64 * sc
    qcv = np.zeros((64, 2), dtype=np.float32)
    qcv[:, 0] = sc
    qcv[:, 1] = zb
    qc128 = np.tile(qcv, (2, 1))

    in_maps = []
    for k in range(8):
        xs = np.empty((NBLK, 128, 4096), dtype=bf16)
        for b in range(NBLK):
            for pk in range(2):
                r0 = k * RC + 16 * b + 2 * pk
                # [16 rows, C, W] -> [g=4, q=4, c=32, w=512] -> [q,c,g,w]
                blk = xpad[r0:r0 + 16].reshape(4, 4, CIN, W)
                xs[b, :, pk * 2048:(pk + 1) * 2048] = \
                    blk.transpose(1, 2, 0, 3).reshape(128, 2048)
        in_maps.append({'xslab': xs, 'wgt': wgt_b, 'qc': qc128})
    return in_maps


def host_finish(results):
    """Per-core [16, 128, 2*WO] int8 -> [1, 510, 510, 64] NHWC.
    out[hg, 64a+co, pk*WO+w] = pixel (h = 4*hg + 2*pk + a, w, co)."""
    full = np.zeros((8 * RC, WO, COUT), dtype=np.int8)
    for k, r in enumerate(results):
        o = r['out'].reshape(NHG, 2, COUT, 2, WO)           # [hg, a, co, pk, w]
        o = np.transpose(o, (0, 3, 1, 4, 2))                # [hg, pk, a, w, co]
        full[k * RC:(k + 1) * RC] = o.reshape(RC, WO, COUT)
    return np.ascontiguousarray(full[:WO])[None]


def run(inputs, n_cores=8, **kw):
    nc = build_nc(n_cores)
    in_maps = host_prepare(**inputs)[:n_cores]
    res = run_bass_kernel_spmd(nc, in_maps, core_ids=list(range(n_cores)), **kw)
    return host_finish(res.results), res


_CACHED_NC = None


def kernel(x, filt, bias, q_mantissa, exponent, output_zero_point):
    global _CACHED_NC
    if _CACHED_NC is None:
        _CACHED_NC = build_nc(8)
    in_maps = host_prepare(x, filt, bias, q_mantissa, exponent, output_zero_point)
    res = run_bass_kernel_spmd(_CACHED_NC, in_maps, core_ids=list(range(8)))
    return host_finish(res.results)
